# revision 42
# baseline (speedup 1.0000x reference)
"""Trainium2 Bass kernel for nn_CausalSelfAttention_56925496541402.

Sliding-window (1024) causal self-attention with rotary embedding,
rms-norm on q/k, and a value-embedding (VE) sigmoid gate. B=1, T=4096,
8 heads x 128 head_dim, n_embd=1024.

Sharding: one head per NeuronCore (8 cores). Each core computes its
head's q/k/v projections, rope+rmsnorm, windowed attention, and its
head's slice of the output projection; the host sums the 8 partial
[4096,1024] outputs (row-block contraction of c_proj).

v2 structure (single merged loop, one ACT table set after phase 0):
  - per 512-token block b: projections(b) -> norm/rope(b) -> attention
    of block b-1 -> v-transpose assembly(b). PE never waits a phase
    boundary; attention(b-1) inputs were finished one iteration ago.
  - rmsnorm rsqrt = exp(-0.5*ln(ms)) using the natural_log_exp ACT set
    (same set serves the softmax exp -> no table thrash). Rope preserves
    norms, so sumsq is taken on the pre-rope projection output.
  - k-side norm is folded into the softmax exp as a per-partition scale:
    rs_k is computed TRANSPOSED (tokens on partitions) via tiny
    [128tok,1] sumsq matmuls, and exp uses scale=rs_k[j]/sqrt(D),
    bias=-4. The k slab stays unnormalized.
  - causal/low-edge masks are additive -30000 matmuls (identity
    stationary) accumulated straight into the S psum region: the
    S->exp->PV chain has no DVE/Pool hop.
  - v is produced pre-transposed ([token,dim]) by transposed projection
    matmuls (stationary = x chunk), and assembled with one
    scalar_tensor_tensor per 128-token subblock:
    vsl = vetT*gateT + vpT. The VE gate is computed transposed too
    (32 tiny matmuls + ONE sigmoid op).
  - PSUM budget exactly 8 banks: proj ring 2 (also vpT), S ring 2
    (also msq/kt scratch), yp 1, dp 1, outproj ring 2.

fp16 data path; matmul accumulation and softmax stats f32.
exp(S*scale - 4) keeps attention weights inside fp16 range.
"""
import sys
sys.path.insert(0, "/opt/trn_rl_repo")
import math
import numpy as np

T = 4096
TB = 512           # t-block width
NBLK = T // TB
D = 128            # head dim
C = 1024           # n_embd
NCO = C // 128     # embed chunks
WIN = 1024
NCORES = 8
SCALE = 1.0 / math.sqrt(D)
EXP_BIAS = -4.0    # exp(S*scale - 4): fp16-safe range, cancels in normalize
MASK_NEG = -30000.0

_prog_cache = {}
_last_in_maps = None


def _chunk_list(b):
    """Key chunks for query block b (i0=512b): (j0, mask_idx, lo, hi).

    [lo, hi) is the computed query range; mask_idx selects the additive
    boundary mask (0=low-edge pattern, 1=causal pattern) applied on the
    128-wide triangle boundary [mlo, mlo+128) inside it. The first chunk
    covers [0, 512) so its start=True matmul initializes every psum col.
    """
    i0 = TB * b
    out = []
    for c in range(4):           # full chunks (emitted first)
        j0 = i0 - 512 + 128 * c
        if j0 >= 0:
            out.append((j0, None, 0, 0, 512))
    for c in range(4):           # causal chunks: visible i in [128c, 512)
        j0 = i0 + 128 * c
        out.append((j0, 1, 128 * c, 128 * c, 512))
    for c in range(4):           # low-edge chunks: visible i in [0, 128c+128)
        j0 = i0 - 1024 + 128 * c
        if j0 >= 0:
            out.append((j0, 0, 128 * c, 0, 128 * (c + 1)))
    if b == 0:
        assert out[0][3] == 0 and out[0][4] == 512
    return out


def _patch_act_tables(bacc):
    """Make the ATL-insertion pass resolve ln AND exp to the single
    natural_log_exp_and_others set: empty out the competing sets (list
    positions preserved so act_func_set_id still indexes act_info.json).
    Only the build-time pass sees this; the interpreter reads the real
    tables from hw_specs directly."""
    from concourse.hw_specs import get_activation_tables
    blocked = {"exp_and_others", "natural_log", "exp_and_friends"}

    def patched(arch):
        tabs = dict(get_activation_tables(arch))
        for name in list(tabs):
            if name in blocked:
                tabs[name] = set()
        return tabs

    bacc.get_activation_tables = patched


def _build_program(nreps=1):
    import concourse.bass as bass
    import concourse.mybir as mybir
    import concourse.tile as tile
    from concourse import bacc
    from concourse.masks import make_identity
    _patch_act_tables(bacc)

    F32 = mybir.dt.float32
    F16 = mybir.dt.float16
    AF = mybir.ActivationFunctionType
    MUL = mybir.AluOpType.mult
    ADD = mybir.AluOpType.add
    ts = bass.ts

    nc = bacc.Bacc("TRN2", target_bir_lowering=False, debug=False,
                   enable_asserts=True, num_devices=1)

    # x_pre[p, co*T + t] = x[t, co*128+p]: per-partition contiguous lines
    xT = nc.dram_tensor("xT", [128, NCO * T], F16, kind="ExternalInput").ap()
    cc_d = nc.dram_tensor("cc", [D, T], F16, kind="ExternalInput").ap()
    ss_d = nc.dram_tensor("ssw", [D, T], F16, kind="ExternalInput").ap()
    # veT2[p, m, d] = 2*ve[m*128+p, d] (token-major: transposed vs v1)
    ve2_d = nc.dram_tensor("veT2", [128, (T // 128) * D], F16,
                           kind="ExternalInput").ap()
    wq_d = nc.dram_tensor("wq", [128, C], F16, kind="ExternalInput").ap()
    wk_d = nc.dram_tensor("wk", [128, C], F16, kind="ExternalInput").ap()
    wv_d = nc.dram_tensor("wv", [128, C], F16, kind="ExternalInput").ap()
    wp_d = nc.dram_tensor("wp", [D, C], F16, kind="ExternalInput").ap()
    wg_d = nc.dram_tensor("wg", [32, 128], F16, kind="ExternalInput").ap()
    # amk[p, m, i]: additive masks, m=0 low-edge (0 iff i<p), m=1 causal
    amk_d = nc.dram_tensor("amk", [128, 2 * 128], F16, kind="ExternalInput").ap()
    on_d = nc.dram_tensor("ones", [128, 128], F16, kind="ExternalInput").ap()
    out_d = nc.dram_tensor("out", [T, C], F16, kind="ExternalOutput").ap()

    xT3 = xT.rearrange("p (co t) -> p co t", co=NCO)
    LNBIAS = -0.5 * math.log(D)   # folds 1/sqrt(D) into rs_kT

    with tile.TileContext(nc) as tc:
        with tc.tile_pool(name="const", bufs=1) as cst:
            wq_sb = cst.tile([128, NCO, D], F16, tag="wq")
            wk_sb = cst.tile([128, NCO, D], F16, tag="wk")
            wv_sb = cst.tile([128, NCO, D], F16, tag="wv")
            wp_sb = cst.tile([128, C], F16, tag="wp")
            wg_sb = cst.tile([32, 128], F16, tag="wg")
            amk_sb = cst.tile([128, 2, 128], F16, tag="amk")
            on_sb = cst.tile([128, 128], F16, tag="on")
            ident = cst.tile([128, 128], F16, tag="ident")
            eb = cst.tile([128, 1], F32, tag="eb")
            eps = cst.tile([128, 1], F32, tag="eps")
            lnb = cst.tile([128, 1], F32, tag="lnb")
            zb = cst.tile([128, 1], F32, tag="zb")
            cc_sb = cst.tile([128, T], F16, tag="cc")
            ss_sb = cst.tile([128, T], F16, tag="ssw")
            ve2 = cst.tile([128, T // 128, D], F16, tag="ve2")
            gateT = cst.tile([128, T // 128], F16, tag="gateT")
            rskT = cst.tile([128, T // 128], F32, tag="rskT")
            qTn = cst.tile([128, T], F16, tag="qTn")
            kTn = cst.tile([128, T], F16, tag="kTn")
            vsl = cst.tile([128, T // 128, D], F16, tag="vsl")
            x32 = cst.tile([32, T], F16, tag="x32")

            make_identity(nc, ident[:])
            nc.gpsimd.memset(eb[:], EXP_BIAS)
            nc.gpsimd.memset(eps[:], 1e-6)
            nc.gpsimd.memset(lnb[:], LNBIAS)
            nc.gpsimd.memset(zb[:], 0.0)

            for _rep in range(nreps):
                with tc.tile_pool(name="xp", bufs=3) as xp, \
                     tc.tile_pool(name="sc", bufs=6) as sc, \
                     tc.tile_pool(name="ptp", bufs=6) as ptp, \
                     tc.tile_pool(name="outp", bufs=3) as outp, \
                     tc.tile_pool(name="pps", bufs=2, space="PSUM") as pps, \
                     tc.tile_pool(name="srg", bufs=2, space="PSUM") as srg, \
                     tc.tile_pool(name="yps", bufs=1, space="PSUM") as yps, \
                     tc.tile_pool(name="dps", bufs=1, space="PSUM") as dps, \
                     tc.tile_pool(name="ops", bufs=2, space="PSUM") as ops:

                    # ---- phase 0: wq first; x(0) arrives per-co so proj(0)
                    # starts after ~2 chunks land ----
                    nc.sync.dma_start(
                        wq_sb[:], wq_d.rearrange("p (co d) -> p co d", co=NCO))

                    xnext = []

                    def emit_phase0_dmas():
                        # ordered just-in-time behind x(0)'s per-co DMAs
                        for wd, w_sb in ((wk_d, wk_sb), (wv_d, wv_sb)):
                            nc.sync.dma_start(
                                w_sb[:],
                                wd.rearrange("p (co d) -> p co d", co=NCO))
                        nc.sync.dma_start(cc_sb[:, 0:TB], cc_d[:, 0:TB])
                        nc.sync.dma_start(ss_sb[:, 0:TB], ss_d[:, 0:TB])
                        xnx = xp.tile([128, NCO, 2 * TB], F16, tag="x",
                                      name="xpre")
                        nc.sync.dma_start(xnx[:], xT3[:, :, TB:3 * TB])
                        xnext.append(xnx)
                        nc.sync.dma_start(x32[:], xT3[0:32, 0, :])
                        nc.sync.dma_start(wg_sb[:], wg_d)
                        nc.sync.dma_start(on_sb[:], on_d)
                        nc.sync.dma_start(
                            amk_sb[:], amk_d.rearrange("p (m i) -> p m i", m=2))
                        nc.sync.dma_start(cc_sb[:, TB:T], cc_d[:, TB:T])
                        nc.sync.dma_start(ss_sb[:, TB:T], ss_d[:, TB:T])
                        nc.sync.dma_start(
                            ve2[:], ve2_d.rearrange("p (m d) -> p m d", d=D))
                        nc.sync.dma_start(wp_sb[:], wp_d)

                    def emit_gate():
                        # gateT via 32 tiny transposed matmuls + ONE sigmoid
                        g_ps = srg.tile([128, 512], F32, tag="s")
                        for m in range(T // 128):
                            nc.tensor.matmul(g_ps[:, m:m + 1],
                                             x32[:, ts(m, 128)], wg_sb[:, 0:1],
                                             start=True, stop=True)
                        nc.scalar.activation(gateT[:], g_ps[:, 0:T // 128],
                                             AF.Sigmoid)

                    # ---- main merged loop ----
                    pending = None          # outproj deferred one block

                    def emit_outproj_tcc(yt, i0, tcc):
                        ost = outp.tile([128, 1024], F16, tag="ost")
                        for hh in range(2):
                            op2 = ops.tile([128, 512], F32, tag="op")
                            nc.tensor.matmul(op2[:],
                                             yt[:, ts(tcc, 128)],
                                             wp_sb[:, ts(hh, 512)],
                                             start=True, stop=True)
                            (nc.scalar.copy if hh == 0 else
                             nc.vector.tensor_copy)(ost[:, ts(hh, 512)],
                                                    op2[:])
                        nc.sync.dma_start(
                            out_d[i0 + 128 * tcc:i0 + 128 * (tcc + 1), :],
                            ost[:])

                    def emit_outproj_next():
                        # emit one remaining tcc of the deferred outproj
                        nonlocal pending
                        if pending is None:
                            return
                        yt, i0, tcc = pending
                        emit_outproj_tcc(yt, i0, tcc)
                        pending = [yt, i0, tcc + 1] if tcc < 3 else None

                    def emit_outproj_flush():
                        while pending is not None:
                            emit_outproj_next()

                    def attention_begin(b, final=False):
                        return {
                            "b": b, "i0": TB * b, "chunks": _chunk_list(b),
                            "final": final, "pts": {}, "pos": 0,
                            "yp": yps.tile([128, TB], F32, tag="y", name="yp"),
                            "dp": dps.tile([128, TB], F32, tag="d", name="dp"),
                            "LAG": 3 if final else 1,
                        }

                    def attention_steps(ast, upto):
                        nonlocal pending
                        chunks, n = ast["chunks"], len(ast["chunks"])
                        i0, pts, LAG = ast["i0"], ast["pts"], ast["LAG"]
                        yp, dp, final = ast["yp"], ast["dp"], ast["final"]
                        for step in range(ast["pos"], min(upto, n + LAG)):
                            # spread the deferred outproj over steps 3/5/7/9 so
                            # its psum->sbuf copies don't wedge the exp stream
                            if step in (3, 5, 7, 9):
                                emit_outproj_next()
                            if step < n:
                                j0, mi, mlo, lo, hi = chunks[step]
                                w = hi - lo
                                # the final attention has no proj work to
                                # hide exp latency: borrow the idle proj ring
                                ring = pps if (final and step % 2) else srg
                                sp = ring.tile([128, 512], F32,
                                               tag="proj" if ring is pps else "s")
                                nc.tensor.matmul(sp[:, 0:w],
                                                 kTn[:, j0:j0 + 128],
                                                 qTn[:, i0 + lo:i0 + hi],
                                                 start=True, stop=(mi is None))
                                if mi is not None:
                                    nc.tensor.matmul(
                                        sp[:, mlo - lo:mlo - lo + 128],
                                        ident[:], amk_sb[:, mi, :],
                                        start=False, stop=True,
                                        skip_group_check=True)
                                pt = ptp.tile([128, 512], F16, tag="pt")
                                nc.scalar.activation(
                                    pt[:, 0:w], sp[:, 0:w], AF.Exp,
                                    scale=rskT[:, j0 // 128:j0 // 128 + 1],
                                    bias=eb[:])
                                pts[step] = pt
                            idx = step - LAG
                            if 0 <= idx < n:
                                j0, mi, mlo, lo, hi = chunks[idx]
                                w = hi - lo
                                pt = pts.pop(idx)
                                st, sp_ = (idx == 0), (idx == n - 1)
                                nc.tensor.matmul(dp[:, lo:hi], on_sb[:],
                                                 pt[:, 0:w], start=st, stop=sp_)
                                nc.tensor.matmul(yp[:, lo:hi],
                                                 vsl[:, j0 // 128, :],
                                                 pt[:, 0:w], start=st, stop=sp_)
                        ast["pos"] = min(upto, n + LAG)

                    def attention_finish(ast):
                        nonlocal pending
                        attention_steps(ast, 99)
                        emit_outproj_flush()
                        rc = sc.tile([128, TB], F32, tag="rc")
                        nc.vector.reciprocal_approx_fast(rc[:], ast["dp"][:])
                        yt = sc.tile([128, TB], F16, tag="yt")
                        nc.vector.tensor_tensor(yt[:], ast["yp"][:], rc[:], MUL)
                        pending = [yt, ast["i0"], 0]

                    x_sb2 = None
                    ast = None
                    for b in range(NBLK + 1):
                        if b >= 1:
                            ast = attention_begin(b - 1, final=(b == NBLK))
                        if b < NBLK:
                            sl = ts(b, TB)
                            if b == 0:
                                x_sb2 = xp.tile([128, NCO, 2 * TB], F16, tag="x")
                                for co in range(NCO):
                                    nc.sync.dma_start(x_sb2[:, co, 0:TB],
                                                      xT3[:, co, 0:TB])
                                x_sb = x_sb2[:, :, 0:TB]
                                emit_phase0_dmas()
                            elif b % 2 == 1:
                                x_sb2 = xnext[(b - 1) // 2]
                                x_sb = x_sb2[:, :, 0:TB]
                                # prefetch two blocks, two iterations ahead
                                if b + 2 < NBLK:
                                    nb = min(2, NBLK - (b + 2))
                                    xnx2 = xp.tile([128, NCO, 2 * TB], F16,
                                                   tag="x", name="xpre2")
                                    nc.sync.dma_start(
                                        xnx2[:, :, 0:TB * nb],
                                        xT3[:, :, TB * (b + 2):TB * (b + 2 + nb)])
                                    xnext.append(xnx2)
                            else:
                                x_sb = x_sb2[:, :, TB:2 * TB]

                            # q/k projections (wide), v projection (transposed)
                            up_q = pps.tile([128, TB], F32, tag="proj")
                            for co in range(NCO):
                                nc.tensor.matmul(up_q[:], wq_sb[:, co, :],
                                                 x_sb[:, co, :],
                                                 start=(co == 0),
                                                 stop=(co == NCO - 1))
                            u16q = sc.tile([128, TB], F16, tag="u16q")
                            nc.scalar.copy(u16q[:], up_q[:])
                            up_k = pps.tile([128, TB], F32, tag="proj")
                            for co in range(NCO):
                                nc.tensor.matmul(up_k[:], wk_sb[:, co, :],
                                                 x_sb[:, co, :],
                                                 start=(co == 0),
                                                 stop=(co == NCO - 1))
                            u16k = sc.tile([128, TB], F16, tag="u16k")
                            nc.scalar.copy(u16k[:], up_k[:])
                            # vpT[tok, d] per 128-token subblock (transposed)
                            vpT = pps.tile([128, TB], F32, tag="proj")
                            for m in range(4):
                                for co in range(NCO):
                                    nc.tensor.matmul(
                                        vpT[:, ts(m, 128)],
                                        x_sb[:, co, ts(m, 128)],
                                        wv_sb[:, co, :],
                                        start=(co == 0), stop=(co == NCO - 1))

                            # sumsq (pre-rope == post-rope; rope is a rotation)
                            sq_q = sc.tile([128, TB], F16, tag="sq")
                            nc.vector.tensor_tensor(sq_q[:], u16q[:], u16q[:], MUL)
                            sq_k = sc.tile([128, TB], F16, tag="sqk")
                            nc.vector.tensor_tensor(sq_k[:], u16k[:], u16k[:], MUL)

                            # rope q (unnormalized for now), rope k
                            y_q = None
                            for u16, qside in ((u16q, True), (u16k, False)):
                                t1 = sc.tile([128, TB], F16, tag="t1")
                                nc.vector.tensor_tensor(t1[:], u16[:],
                                                        cc_sb[:, sl], MUL)
                                p = sc.tile([128, TB], F16, tag="p")
                                nc.vector.tensor_tensor(p[:], u16[:],
                                                        ss_sb[:, sl], MUL)
                                pr = sc.tile([128, TB], F16, tag="pr")
                                nc.vector.tensor_copy(pr[0:64, :], p[64:128, :])
                                nc.vector.tensor_copy(pr[64:128, :], p[0:64, :])
                                if qside:
                                    y_q = sc.tile([128, TB], F16, tag="yq")
                                    nc.vector.tensor_tensor(y_q[:], t1[:], pr[:],
                                                            ADD)
                                else:
                                    nc.vector.tensor_tensor(kTn[:, sl], t1[:],
                                                            pr[:], ADD)

                            # v assembly: vsl = ve2*gateT + vpT  (transposed)
                            for m in range(4):
                                mm = 4 * b + m
                                nc.vector.scalar_tensor_tensor(
                                    vsl[:, mm, :], ve2[:, mm, :],
                                    gateT[:, mm:mm + 1], vpT[:, ts(m, 128)],
                                    op0=MUL, op1=ADD)

                        if ast is not None:
                            attention_finish(ast)
                            ast = None

                        if b < NBLK:
                            # norm chain AFTER attention(b-1): its ACT ops sit
                            # behind the exp stream, its results are consumed
                            # next iteration. rs = exp(-0.5*ln(ms/D + eps)).
                            msq = srg.tile([128, 512], F32, tag="s")
                            nc.tensor.matmul(msq[:], on_sb[:], sq_q[:],
                                             start=True, stop=True)
                            kt = srg.tile([128, 512], F32, tag="s")
                            for m in range(4):
                                nc.tensor.matmul(kt[:, m:m + 1],
                                                 sq_k[:, ts(m, 128)],
                                                 on_sb[:, 0:1],
                                                 start=True, stop=True)
                            Lq = sc.tile([128, TB], F16, tag="Lq")
                            nc.scalar.activation(Lq[:], msq[:], AF.Ln,
                                                 scale=1.0 / D, bias=eps[:])
                            rsq = sc.tile([128, TB], F16, tag="rsq")
                            nc.scalar.activation(rsq[:], Lq[:], AF.Exp,
                                                 scale=-0.5, bias=zb[:])
                            Lk = sc.tile([128, 4], F32, tag="Lk")
                            nc.scalar.activation(Lk[:], kt[:, 0:4], AF.Ln,
                                                 scale=1.0 / D, bias=eps[:])
                            nc.scalar.activation(rskT[:, 4 * b:4 * b + 4],
                                                 Lk[:], AF.Exp,
                                                 scale=-0.5, bias=lnb[:])
                            nc.vector.tensor_tensor(qTn[:, sl], y_q[:],
                                                    rsq[:], MUL)
                            if b == 0:
                                emit_gate()
                    emit_outproj_flush()

    nc.finalize()
    return nc


def _w_pre(w):
    # w_pre[p, co*128 + d] = w[co*128+p, d]
    return np.ascontiguousarray(
        w.reshape(NCO, 128, D).transpose(1, 0, 2).reshape(128, C)
    ).astype(np.float16)


def _build_addmasks():
    jj = np.arange(128)[:, None]
    ii = np.arange(128)[None, :]
    mk = np.zeros((128, 2, 128), dtype=np.float16)
    mk[:, 0, :] = np.where(ii < jj, 0.0, MASK_NEG)      # low-edge
    mk[:, 1, :] = np.where(ii >= jj, 0.0, MASK_NEG)     # causal
    return mk.reshape(128, 256)


def kernel(x, ve, cos, sin, wq, wk, wv, w_gate, w_proj, window_size):
    from concourse.bass_utils import run_bass_kernel_spmd

    assert int(np.asarray(window_size)) == WIN
    x = np.asarray(x, dtype=np.float32)
    ve = np.asarray(ve, dtype=np.float32)
    cos = np.asarray(cos, dtype=np.float32).reshape(T, 64)
    sin = np.asarray(sin, dtype=np.float32).reshape(T, 64)
    wq = np.asarray(wq, dtype=np.float32)
    wk = np.asarray(wk, dtype=np.float32)
    wv = np.asarray(wv, dtype=np.float32)
    w_gate = np.asarray(w_gate, dtype=np.float32)
    w_proj = np.asarray(w_proj, dtype=np.float32)
    assert x.shape == (1, T, C) and ve.shape == (1, T, C)

    if "nc" not in _prog_cache:
        _prog_cache["nc"] = _build_program()
    nc = _prog_cache["nc"]

    # x_pre[p, co*T + t] = x[t, co*128+p]
    xT_h = np.ascontiguousarray(
        x[0].T.reshape(NCO, 128, T).transpose(1, 0, 2).reshape(128, NCO * T)
    ).astype(np.float16)
    cosT, sinT = cos.T, sin.T                                # [64, T]
    cc = np.concatenate([cosT, cosT], axis=0).astype(np.float16)
    # p[d] = u[d]*ssw[d]; y[d] = u[d]*cc[d] + p[swap(d)]  => ssw = [-sinT; sinT]
    ssw = np.concatenate([-sinT, sinT], axis=0).astype(np.float16)
    amk = _build_addmasks()
    ones = np.ones((128, 128), dtype=np.float16)

    in_maps = []
    for h in range(NCORES):
        d = D * h
        # veT2[p, m*128 + dd] = 2*ve[m*128+p, d+dd]
        ve2 = (2.0 * ve[0][:, d:d + D]).reshape(T // 128, 128, D)
        ve2 = np.ascontiguousarray(ve2.transpose(1, 0, 2).reshape(128, -1))
        in_maps.append({
            "xT": xT_h,
            "cc": cc,
            "ssw": ssw,
            "veT2": ve2.astype(np.float16),
            "wq": _w_pre(wq[:, d:d + D]),
            "wk": _w_pre(wk[:, d:d + D]),
            "wv": _w_pre(wv[:, d:d + D]),
            "wp": np.ascontiguousarray(w_proj[d:d + D, :]).astype(np.float16),
            "wg": np.tile(w_gate[:, h:h + 1], (1, 128)).astype(np.float16),
            "amk": amk,
            "ones": ones,
        })

    global _last_in_maps
    _last_in_maps = in_maps
    res = run_bass_kernel_spmd(nc, in_maps, core_ids=list(range(NCORES)))
    out = np.zeros((T, C), dtype=np.float32)
    for h in range(NCORES):
        out += res.results[h]["out"].astype(np.float32)
    return out.reshape(1, T, C)


# revision 48
# speedup vs baseline: 1.2566x; 1.2566x over previous
"""Trainium2 Bass kernel for nn_CausalSelfAttention_56925496541402.

Sliding-window (1024) causal self-attention with rotary embedding,
rms-norm on q/k, and a value-embedding (VE) sigmoid gate. B=1, T=4096,
8 heads x 128 head_dim, n_embd=1024.

Sharding: one head per NeuronCore (8 cores). Each core computes its
head's q/k/v projections, rope+rmsnorm, windowed attention, and its
head's slice of the output projection; the host sums the 8 partial
[4096,1024] outputs (row-block contraction of c_proj).

v2 structure (single merged loop, one ACT table set after phase 0):
  - per 512-token block b: projections(b) -> norm/rope(b) -> attention
    of block b-1 -> v-transpose assembly(b). PE never waits a phase
    boundary; attention(b-1) inputs were finished one iteration ago.
  - rmsnorm rsqrt = exp(-0.5*ln(ms)) using the natural_log_exp ACT set
    (same set serves the softmax exp -> no table thrash). Rope preserves
    norms, so sumsq is taken on the pre-rope projection output.
  - k-side norm is folded into the softmax exp as a per-partition scale:
    rs_k is computed TRANSPOSED (tokens on partitions) via tiny
    [128tok,1] sumsq matmuls, and exp uses scale=rs_k[j]/sqrt(D),
    bias=-4. The k slab stays unnormalized.
  - causal/low-edge masks are additive -30000 matmuls (identity
    stationary) accumulated straight into the S psum region: the
    S->exp->PV chain has no DVE/Pool hop.
  - v is produced pre-transposed ([token,dim]) by transposed projection
    matmuls (stationary = x chunk), and assembled with one
    scalar_tensor_tensor per 128-token subblock:
    vsl = vetT*gateT + vpT. The VE gate is computed transposed too
    (32 tiny matmuls + ONE sigmoid op).
  - PSUM budget exactly 8 banks: proj ring 2 (also vpT), S ring 2
    (also msq/kt scratch), yp 1, dp 1, outproj ring 2.

fp16 data path; matmul accumulation and softmax stats f32.
exp(S*scale - 4) keeps attention weights inside fp16 range.
"""
import sys
sys.path.insert(0, "/opt/trn_rl_repo")
import math
import numpy as np

T = 4096
TB = 512           # t-block width
NBLK = T // TB
D = 128            # head dim
C = 1024           # n_embd
NCO = C // 128     # embed chunks
WIN = 1024
NCORES = 8
SCALE = 1.0 / math.sqrt(D)
EXP_BIAS = -4.0    # exp(S*scale - 4): fp16-safe range, cancels in normalize
MASK_NEG = -30000.0

_prog_cache = {}
_last_in_maps = None


def _chunk_list(b):
    """Key chunks for query block b (i0=512b): (j0, mask_idx, lo, hi).

    [lo, hi) is the computed query range; mask_idx selects the additive
    boundary mask (0=low-edge pattern, 1=causal pattern) applied on the
    128-wide triangle boundary [mlo, mlo+128) inside it. The first chunk
    covers [0, 512) so its start=True matmul initializes every psum col.
    """
    i0 = TB * b
    out = []
    for c in range(4):           # full chunks (emitted first)
        j0 = i0 - 512 + 128 * c
        if j0 >= 0:
            out.append((j0, None, 0, 0, 512))
    for c in range(4):           # causal chunks: visible i in [128c, 512)
        j0 = i0 + 128 * c
        out.append((j0, 1, 128 * c, 128 * c, 512))
    for c in range(4):           # low-edge chunks: visible i in [0, 128c+128)
        j0 = i0 - 1024 + 128 * c
        if j0 >= 0:
            out.append((j0, 0, 128 * c, 0, 128 * (c + 1)))
    if b == 0:
        assert out[0][3] == 0 and out[0][4] == 512
    return out


def _patch_act_tables(bacc):
    """Make the ATL-insertion pass resolve ln AND exp to the single
    natural_log_exp_and_others set: empty out the competing sets (list
    positions preserved so act_func_set_id still indexes act_info.json).
    Only the build-time pass sees this; the interpreter reads the real
    tables from hw_specs directly."""
    from concourse.hw_specs import get_activation_tables
    blocked = {"exp_and_others", "natural_log", "exp_and_friends"}

    def patched(arch):
        tabs = dict(get_activation_tables(arch))
        for name in list(tabs):
            if name in blocked:
                tabs[name] = set()
        return tabs

    bacc.get_activation_tables = patched


def _build_program(nreps=1):
    import concourse.bass as bass
    import concourse.mybir as mybir
    import concourse.tile as tile
    from concourse import bacc
    from concourse.masks import make_identity
    _patch_act_tables(bacc)

    F32 = mybir.dt.float32
    F16 = mybir.dt.float16
    AF = mybir.ActivationFunctionType
    MUL = mybir.AluOpType.mult
    ADD = mybir.AluOpType.add
    ts = bass.ts

    nc = bacc.Bacc("TRN2", target_bir_lowering=False, debug=False,
                   enable_asserts=True, num_devices=1)

    # x_pre[p, co*T + t] = x[t, co*128+p]: per-partition contiguous lines
    xT = nc.dram_tensor("xT", [128, NCO * T], F16, kind="ExternalInput").ap()
    cc_d = nc.dram_tensor("cc", [D, T], F16, kind="ExternalInput").ap()
    ss_d = nc.dram_tensor("ssw", [D, T], F16, kind="ExternalInput").ap()
    # veT2[p, m, d] = 2*ve[m*128+p, d] (token-major: transposed vs v1)
    ve2_d = nc.dram_tensor("veT2", [128, (T // 128) * D], F16,
                           kind="ExternalInput").ap()
    wq_d = nc.dram_tensor("wq", [128, C], F16, kind="ExternalInput").ap()
    wk_d = nc.dram_tensor("wk", [128, C], F16, kind="ExternalInput").ap()
    wv_d = nc.dram_tensor("wv", [128, C], F16, kind="ExternalInput").ap()
    wp_d = nc.dram_tensor("wp", [D, C], F16, kind="ExternalInput").ap()
    wg_d = nc.dram_tensor("wg", [32, 128], F16, kind="ExternalInput").ap()
    # amk[p, m, i]: additive masks, m=0 low-edge (0 iff i<p), m=1 causal
    amk_d = nc.dram_tensor("amk", [128, 2 * 128], F16, kind="ExternalInput").ap()
    on_d = nc.dram_tensor("ones", [128, 128], F16, kind="ExternalInput").ap()
    out_d = nc.dram_tensor("out", [T, C], F16, kind="ExternalOutput").ap()

    xT3 = xT.rearrange("p (co t) -> p co t", co=NCO)
    LNBIAS = -0.5 * math.log(D)   # folds 1/sqrt(D) into rs_kT

    with tile.TileContext(nc) as tc:
        with tc.tile_pool(name="const", bufs=1) as cst:
            wq_sb = cst.tile([128, NCO, D], F16, tag="wq")
            wk_sb = cst.tile([128, NCO, D], F16, tag="wk")
            wv_sb = cst.tile([128, NCO, D], F16, tag="wv")
            wp_sb = cst.tile([128, C], F16, tag="wp")
            wg_sb = cst.tile([32, 128], F16, tag="wg")
            amk_sb = cst.tile([128, 2, 128], F16, tag="amk")
            on_sb = cst.tile([128, 128], F16, tag="on")
            ident = cst.tile([128, 128], F16, tag="ident")
            eb = cst.tile([128, 1], F32, tag="eb")
            eps = cst.tile([128, 1], F32, tag="eps")
            lnb = cst.tile([128, 1], F32, tag="lnb")
            zb = cst.tile([128, 1], F32, tag="zb")
            cc_sb = cst.tile([128, T], F16, tag="cc")
            ss_sb = cst.tile([128, T], F16, tag="ssw")
            ve2 = cst.tile([128, T // 128, D], F16, tag="ve2")
            gateT = cst.tile([128, T // 128], F16, tag="gateT")
            rskT = cst.tile([128, T // 128], F32, tag="rskT")
            qTn = cst.tile([128, T], F16, tag="qTn")
            kTn = cst.tile([128, T], F16, tag="kTn")
            vsl = cst.tile([128, T // 128, D], F16, tag="vsl")
            x32 = cst.tile([32, T], F16, tag="x32")

            make_identity(nc, ident[:])
            nc.gpsimd.memset(eb[:], EXP_BIAS)
            nc.gpsimd.memset(eps[:], 1e-6)
            nc.gpsimd.memset(lnb[:], LNBIAS)
            nc.gpsimd.memset(zb[:], 0.0)

            for _rep in range(nreps):
                with tc.tile_pool(name="xp", bufs=3) as xp, \
                     tc.tile_pool(name="sc", bufs=2) as sc, \
                     tc.tile_pool(name="ptp", bufs=6) as ptp, \
                     tc.tile_pool(name="outp", bufs=3) as outp, \
                     tc.tile_pool(name="pps", bufs=2, space="PSUM") as pps, \
                     tc.tile_pool(name="srg", bufs=2, space="PSUM") as srg, \
                     tc.tile_pool(name="yps", bufs=1, space="PSUM") as yps, \
                     tc.tile_pool(name="dps", bufs=1, space="PSUM") as dps, \
                     tc.tile_pool(name="ops", bufs=2, space="PSUM") as ops:

                    # ---- phase 0: wq first; x(0) arrives per-co so proj(0)
                    # starts after ~2 chunks land ----
                    nc.sync.dma_start(
                        wq_sb[:], wq_d.rearrange("p (co d) -> p co d", co=NCO))

                    xnext = []

                    def emit_phase0_dmas():
                        # ordered just-in-time behind x(0)'s per-co DMAs
                        for wd, w_sb in ((wk_d, wk_sb), (wv_d, wv_sb)):
                            nc.sync.dma_start(
                                w_sb[:],
                                wd.rearrange("p (co d) -> p co d", co=NCO))
                        nc.sync.dma_start(cc_sb[:, 0:TB], cc_d[:, 0:TB])
                        nc.sync.dma_start(ss_sb[:, 0:TB], ss_d[:, 0:TB])
                        xnx = xp.tile([128, NCO, 2 * TB], F16, tag="x",
                                      name="xpre")
                        nc.sync.dma_start(xnx[:], xT3[:, :, TB:3 * TB])
                        xnext.append(xnx)
                        nc.sync.dma_start(x32[:], xT3[0:32, 0, :])
                        nc.sync.dma_start(wg_sb[:], wg_d)
                        nc.sync.dma_start(on_sb[:], on_d)
                        nc.sync.dma_start(
                            amk_sb[:], amk_d.rearrange("p (m i) -> p m i", m=2))
                        nc.sync.dma_start(cc_sb[:, TB:T], cc_d[:, TB:T])
                        nc.sync.dma_start(ss_sb[:, TB:T], ss_d[:, TB:T])
                        nc.sync.dma_start(
                            ve2[:], ve2_d.rearrange("p (m d) -> p m d", d=D))
                        nc.sync.dma_start(wp_sb[:], wp_d)

                    def emit_gate():
                        # gateT via 32 tiny transposed matmuls + ONE sigmoid
                        g_ps = srg.tile([128, 512], F32, tag="s")
                        for m in range(T // 128):
                            nc.tensor.matmul(g_ps[:, m:m + 1],
                                             x32[:, ts(m, 128)], wg_sb[:, 0:1],
                                             start=True, stop=True)
                        nc.scalar.activation(gateT[:], g_ps[:, 0:T // 128],
                                             AF.Sigmoid)

                    # ---- main merged loop ----
                    pending = None          # outproj deferred one block

                    def emit_outproj_tcc(yt, i0, tcc):
                        ost = outp.tile([128, 1024], F16, tag="ost")
                        for hh in range(2):
                            op2 = ops.tile([128, 512], F32, tag="op")
                            nc.tensor.matmul(op2[:],
                                             yt[:, ts(tcc, 128)],
                                             wp_sb[:, ts(hh, 512)],
                                             start=True, stop=True)
                            (nc.scalar.copy if hh == 0 else
                             nc.vector.tensor_copy)(ost[:, ts(hh, 512)],
                                                    op2[:])
                        nc.sync.dma_start(
                            out_d[i0 + 128 * tcc:i0 + 128 * (tcc + 1), :],
                            ost[:])

                    def emit_outproj_next():
                        # emit one remaining tcc of the deferred outproj
                        nonlocal pending
                        if pending is None:
                            return
                        yt, i0, tcc = pending
                        emit_outproj_tcc(yt, i0, tcc)
                        pending = [yt, i0, tcc + 1] if tcc < 3 else None

                    def emit_outproj_flush():
                        while pending is not None:
                            emit_outproj_next()

                    def attention_begin(b, final=False):
                        return {
                            "b": b, "i0": TB * b, "chunks": _chunk_list(b),
                            "final": final, "pts": {}, "pos": 0,
                            "yp": yps.tile([128, TB], F32, tag="y", name="yp"),
                            "dp": dps.tile([128, TB], F32, tag="d", name="dp"),
                            "LAG": 3 if final else 1,
                        }

                    def attention_steps(ast, upto):
                        nonlocal pending
                        chunks, n = ast["chunks"], len(ast["chunks"])
                        i0, pts, LAG = ast["i0"], ast["pts"], ast["LAG"]
                        yp, dp, final = ast["yp"], ast["dp"], ast["final"]
                        for step in range(ast["pos"], min(upto, n + LAG)):
                            # spread the deferred outproj over steps 3/5/7/9 so
                            # its psum->sbuf copies don't wedge the exp stream
                            if step in (3, 5, 7, 9):
                                emit_outproj_next()
                            if step < n:
                                j0, mi, mlo, lo, hi = chunks[step]
                                w = hi - lo
                                # the final attention has no proj work to
                                # hide exp latency: borrow the idle proj ring
                                ring = pps if (final and step % 2) else srg
                                sp = ring.tile([128, 512], F32,
                                               tag="proj" if ring is pps else "s")
                                nc.tensor.matmul(sp[:, 0:w],
                                                 kTn[:, j0:j0 + 128],
                                                 qTn[:, i0 + lo:i0 + hi],
                                                 start=True, stop=(mi is None))
                                if mi is not None:
                                    nc.tensor.matmul(
                                        sp[:, mlo - lo:mlo - lo + 128],
                                        ident[:], amk_sb[:, mi, :],
                                        start=False, stop=True,
                                        skip_group_check=True)
                                pt = ptp.tile([128, 512], F16, tag="pt")
                                nc.scalar.activation(
                                    pt[:, 0:w], sp[:, 0:w], AF.Exp,
                                    scale=rskT[:, j0 // 128:j0 // 128 + 1],
                                    bias=eb[:])
                                pts[step] = pt
                            idx = step - LAG
                            if 0 <= idx < n:
                                j0, mi, mlo, lo, hi = chunks[idx]
                                w = hi - lo
                                pt = pts.pop(idx)
                                st, sp_ = (idx == 0), (idx == n - 1)
                                nc.tensor.matmul(dp[:, lo:hi], on_sb[:],
                                                 pt[:, 0:w], start=st, stop=sp_)
                                nc.tensor.matmul(yp[:, lo:hi],
                                                 vsl[:, j0 // 128, :],
                                                 pt[:, 0:w], start=st, stop=sp_)
                        ast["pos"] = min(upto, n + LAG)

                    def attention_finish(ast):
                        nonlocal pending
                        attention_steps(ast, 99)
                        emit_outproj_flush()
                        rc = sc.tile([128, TB], F32, tag="rc")
                        nc.vector.reciprocal_approx_fast(rc[:], ast["dp"][:])
                        yt = sc.tile([128, TB], F16, tag="yt")
                        nc.vector.tensor_tensor(yt[:], ast["yp"][:], rc[:], MUL)
                        pending = [yt, ast["i0"], 0]

                    x_sb2 = None
                    ast = None
                    for b in range(NBLK + 1):
                        if b >= 1:
                            ast = attention_begin(b - 1, final=(b == NBLK))
                        if b < NBLK:
                            sl = ts(b, TB)
                            if b == 0:
                                x_sb2 = xp.tile([128, NCO, 2 * TB], F16, tag="x")
                                for co in range(NCO):
                                    nc.sync.dma_start(x_sb2[:, co, 0:TB],
                                                      xT3[:, co, 0:TB])
                                x_sb = x_sb2[:, :, 0:TB]
                                emit_phase0_dmas()
                            elif b % 2 == 1:
                                x_sb2 = xnext[(b - 1) // 2]
                                x_sb = x_sb2[:, :, 0:TB]
                                # prefetch two blocks, two iterations ahead
                                if b + 2 < NBLK:
                                    nb = min(2, NBLK - (b + 2))
                                    xnx2 = xp.tile([128, NCO, 2 * TB], F16,
                                                   tag="x", name="xpre2")
                                    nc.sync.dma_start(
                                        xnx2[:, :, 0:TB * nb],
                                        xT3[:, :, TB * (b + 2):TB * (b + 2 + nb)])
                                    xnext.append(xnx2)
                            else:
                                x_sb = x_sb2[:, :, TB:2 * TB]

                            # q/k projections (wide), v projection (transposed)
                            up_q = pps.tile([128, TB], F32, tag="proj")
                            for co in range(NCO):
                                nc.tensor.matmul(up_q[:], wq_sb[:, co, :],
                                                 x_sb[:, co, :],
                                                 start=(co == 0),
                                                 stop=(co == NCO - 1))
                            u16q = sc.tile([128, TB], F16, tag="u16q")
                            nc.scalar.copy(u16q[:], up_q[:])
                            up_k = pps.tile([128, TB], F32, tag="proj")
                            for co in range(NCO):
                                nc.tensor.matmul(up_k[:], wk_sb[:, co, :],
                                                 x_sb[:, co, :],
                                                 start=(co == 0),
                                                 stop=(co == NCO - 1))
                            u16k = sc.tile([128, TB], F16, tag="u16k")
                            nc.scalar.copy(u16k[:], up_k[:])
                            # vpT[tok, d] per 128-token subblock (transposed)
                            vpT = pps.tile([128, TB], F32, tag="proj")
                            for m in range(4):
                                for co in range(NCO):
                                    nc.tensor.matmul(
                                        vpT[:, ts(m, 128)],
                                        x_sb[:, co, ts(m, 128)],
                                        wv_sb[:, co, :],
                                        start=(co == 0), stop=(co == NCO - 1))

                            # sumsq (pre-rope == post-rope; rope is a rotation)
                            sq_q = sc.tile([128, TB], F16, tag="sq")
                            nc.vector.tensor_tensor(sq_q[:], u16q[:], u16q[:], MUL)
                            sq_k = sc.tile([128, TB], F16, tag="sqk")
                            nc.vector.tensor_tensor(sq_k[:], u16k[:], u16k[:], MUL)

                            # rope q (unnormalized for now), rope k
                            y_q = None
                            for u16, qside in ((u16q, True), (u16k, False)):
                                t1 = sc.tile([128, TB], F16, tag="t1")
                                nc.vector.tensor_tensor(t1[:], u16[:],
                                                        cc_sb[:, sl], MUL)
                                p = sc.tile([128, TB], F16, tag="p")
                                nc.vector.tensor_tensor(p[:], u16[:],
                                                        ss_sb[:, sl], MUL)
                                pr = sc.tile([128, TB], F16, tag="pr")
                                nc.vector.tensor_copy(pr[0:64, :], p[64:128, :])
                                nc.vector.tensor_copy(pr[64:128, :], p[0:64, :])
                                if qside:
                                    y_q = sc.tile([128, TB], F16, tag="yq")
                                    nc.vector.tensor_tensor(y_q[:], t1[:], pr[:],
                                                            ADD)
                                else:
                                    nc.vector.tensor_tensor(kTn[:, sl], t1[:],
                                                            pr[:], ADD)

                        if ast is not None:
                            attention_finish(ast)
                            ast = None

                        if b < NBLK:
                            # norm chain AFTER attention(b-1): its ACT ops sit
                            # behind the exp stream, its results are consumed
                            # next iteration. rs = exp(-0.5*ln(ms/D + eps)).
                            msq = srg.tile([128, 512], F32, tag="s")
                            nc.tensor.matmul(msq[:], on_sb[:], sq_q[:],
                                             start=True, stop=True)
                            kt = srg.tile([128, 512], F32, tag="s")
                            for m in range(4):
                                nc.tensor.matmul(kt[:, m:m + 1],
                                                 sq_k[:, ts(m, 128)],
                                                 on_sb[:, 0:1],
                                                 start=True, stop=True)
                            Lq = sc.tile([128, TB], F16, tag="Lq")
                            nc.scalar.activation(Lq[:], msq[:], AF.Ln,
                                                 scale=1.0 / D, bias=eps[:])
                            rsq = sc.tile([128, TB], F16, tag="rsq")
                            nc.scalar.activation(rsq[:], Lq[:], AF.Exp,
                                                 scale=-0.5, bias=zb[:])
                            Lk = sc.tile([128, 4], F32, tag="Lk")
                            nc.scalar.activation(Lk[:], kt[:, 0:4], AF.Ln,
                                                 scale=1.0 / D, bias=eps[:])
                            nc.scalar.activation(rskT[:, 4 * b:4 * b + 4],
                                                 Lk[:], AF.Exp,
                                                 scale=-0.5, bias=lnb[:])
                            nc.vector.tensor_tensor(qTn[:, sl], y_q[:],
                                                    rsq[:], MUL)
                            if b == 0:
                                emit_gate()
                            # v assembly: vsl = ve2*gateT + vpT  (transposed);
                            # must follow emit_gate (gateT write) in program
                            # order, and precede attention(b) next iteration
                            for m in range(4):
                                mm = 4 * b + m
                                nc.vector.scalar_tensor_tensor(
                                    vsl[:, mm, :], ve2[:, mm, :],
                                    gateT[:, mm:mm + 1], vpT[:, ts(m, 128)],
                                    op0=MUL, op1=ADD)
                    emit_outproj_flush()

    nc.finalize()
    return nc


def _w_pre(w):
    # w_pre[p, co*128 + d] = w[co*128+p, d]
    return np.ascontiguousarray(
        w.reshape(NCO, 128, D).transpose(1, 0, 2).reshape(128, C)
    ).astype(np.float16)


def _build_addmasks():
    jj = np.arange(128)[:, None]
    ii = np.arange(128)[None, :]
    mk = np.zeros((128, 2, 128), dtype=np.float16)
    mk[:, 0, :] = np.where(ii < jj, 0.0, MASK_NEG)      # low-edge
    mk[:, 1, :] = np.where(ii >= jj, 0.0, MASK_NEG)     # causal
    return mk.reshape(128, 256)


def kernel(x, ve, cos, sin, wq, wk, wv, w_gate, w_proj, window_size):
    from concourse.bass_utils import run_bass_kernel_spmd

    assert int(np.asarray(window_size)) == WIN
    x = np.asarray(x, dtype=np.float32)
    ve = np.asarray(ve, dtype=np.float32)
    cos = np.asarray(cos, dtype=np.float32).reshape(T, 64)
    sin = np.asarray(sin, dtype=np.float32).reshape(T, 64)
    wq = np.asarray(wq, dtype=np.float32)
    wk = np.asarray(wk, dtype=np.float32)
    wv = np.asarray(wv, dtype=np.float32)
    w_gate = np.asarray(w_gate, dtype=np.float32)
    w_proj = np.asarray(w_proj, dtype=np.float32)
    assert x.shape == (1, T, C) and ve.shape == (1, T, C)

    if "nc" not in _prog_cache:
        _prog_cache["nc"] = _build_program()
    nc = _prog_cache["nc"]

    # x_pre[p, co*T + t] = x[t, co*128+p]
    xT_h = np.ascontiguousarray(
        x[0].T.reshape(NCO, 128, T).transpose(1, 0, 2).reshape(128, NCO * T)
    ).astype(np.float16)
    cosT, sinT = cos.T, sin.T                                # [64, T]
    cc = np.concatenate([cosT, cosT], axis=0).astype(np.float16)
    # p[d] = u[d]*ssw[d]; y[d] = u[d]*cc[d] + p[swap(d)]  => ssw = [-sinT; sinT]
    ssw = np.concatenate([-sinT, sinT], axis=0).astype(np.float16)
    amk = _build_addmasks()
    ones = np.ones((128, 128), dtype=np.float16)

    in_maps = []
    for h in range(NCORES):
        d = D * h
        # veT2[p, m*128 + dd] = 2*ve[m*128+p, d+dd]
        ve2 = (2.0 * ve[0][:, d:d + D]).reshape(T // 128, 128, D)
        ve2 = np.ascontiguousarray(ve2.transpose(1, 0, 2).reshape(128, -1))
        in_maps.append({
            "xT": xT_h,
            "cc": cc,
            "ssw": ssw,
            "veT2": ve2.astype(np.float16),
            "wq": _w_pre(wq[:, d:d + D]),
            "wk": _w_pre(wk[:, d:d + D]),
            "wv": _w_pre(wv[:, d:d + D]),
            "wp": np.ascontiguousarray(w_proj[d:d + D, :]).astype(np.float16),
            "wg": np.tile(w_gate[:, h:h + 1], (1, 128)).astype(np.float16),
            "amk": amk,
            "ones": ones,
        })

    global _last_in_maps
    _last_in_maps = in_maps
    res = run_bass_kernel_spmd(nc, in_maps, core_ids=list(range(NCORES)))
    out = np.zeros((T, C), dtype=np.float32)
    for h in range(NCORES):
        out += res.results[h]["out"].astype(np.float32)
    return out.reshape(1, T, C)
